# revision 73
# baseline (speedup 1.0000x reference)
"""KNN top-16 kernel for Trainium2 (8 NeuronCores, SPMD) — v10 (fp16 tree).

Problem (hardcoded): p1 (4,8192,3) f32, p2 (4,8192,3) f32, lengths1/2 (4,) i32.
Returns (idx int64 (4,8192,16), dists f32 (4,8192,16)) matching
jax.lax.top_k(-sq_dists, 16) semantics with PyTorch3D-style padding.

v10 pipeline per 2048-column PSUM group (per 128-query slot):
  PE   : fp16 hi/lo split matmul (16 contraction rows) -> fp32 PSUM,
          1 cycle/column (vs 4 for fp32), exact to ~1e-5 absolute.
  Act  : cast-copy PSUM fp32 -> SBUF fp16 (the Act engine is otherwise
          idle), feeding
  DVE  : a 3-level pairwise fp16 tensor_max tree (2x DVE mode) + one W=8
          fp16 tensor_reduce -> 64-column bin maxima, ~0.71 ns/elem instead
          of 1.04 for a direct fp32 reduce.
  Per-group recipes ('r1' = direct fp32 PSUM reduce with no Act stage,
  'hc' = Act casts only half and DVE's tree level 1 maxes the PSUM half
  against it, 'tree' = full cast) are assigned per (batch, slot parity) in
  _kind_of to balance Act vs DVE busy time globally AND locally; the
  measured split runs both engines at ~92% occupancy.
  The per-query top-16-bin selection runs on the HOST from the fp16 bins
  (monotone rounding keeps the coverage guarantee: a column among the true
  top-16 has at most 15 bins with a strictly larger bin max, so selecting
  all bins >= the 16th-largest bin value always covers it; rows whose
  tie-set exceeds the 32-bin cap fall back to an exact full-row recompute).
  The host then re-ranks the <=32x64 candidate columns exactly in fp32
  (reference formula + tie-break by lower index).
  Input DMA rides gpsimd's SWDGE queue except the first two groups (each
  dma_start holds its engine ~0.6us and serializes on the global HWDGE).

Sharding: live query tile g of batch n runs on core g%8, slot g//8.
"""

import numpy as np
from functools import lru_cache

N, P1, P2, D, K = 4, 8192, 8192, 3, 16
N_CORES = 8
TILE = 128             # query rows per tile
CHUNK = 512            # matmul free-dim chunk (one PSUM bank)
W = 64                 # columns per bin
GROUP = 2048           # psum group (4 banks)
KROWS = 16             # contraction rows (fp16 split encoding)
def _kind_of(bn, gi, parity):
    """'r1': direct fp32 PSUM reduce (no Act); 'hc': Act casts only the
    second half, DVE's tree level 1 maxes PSUM half vs cast half (same bin
    layout as 'tree'); 'tree': full Act cast + fp16 tree."""
    if bn == 0:
        return "r1" if gi == 1 else "tree"
    if bn == 1:
        return "r1" if gi == 2 else "tree"
    if bn == 2:
        if parity == 1:
            return "r1" if gi == 0 else "tree"
        return "hc" if gi == 0 else "tree"
    return "hc" if gi == 1 else "tree"
BIGM = np.float32(60000.0)   # mask magnitude (fits fp16)
BIN_CAP = 32           # host-side max selected bins per row before slow path


def _plan_of(lengths1, lengths2):
    movw = tuple(-(-int(l) // W) * W for l in lengths2)        # pad to bins
    live = tuple(min(P1 // TILE, -(-int(l) // TILE)) for l in lengths1)
    S = tuple(-(-lv // N_CORES) for lv in live)
    return (movw, live, S)


def _groups_of(wb):
    """[(g0, gw)] covering [0, wb) in GROUP-sized pieces."""
    gs = []
    g0 = 0
    while g0 < wb:
        gw = min(GROUP, wb - g0)
        gs.append((g0, gw))
        g0 += gw
    return gs


def _recipes_of(movw):
    """Per (batch, slot-parity): list of (g0, gw, kind)."""
    out = {}
    for bn in range(N):
        for parity in (0, 1):
            rs = []
            for gi, (g0, gw) in enumerate(_groups_of(movw[bn])):
                if gw == GROUP:
                    kind = _kind_of(bn, gi, parity)
                else:
                    kind = "r1" if gw <= 512 else "tree"
                rs.append((g0, gw, kind))
            out[(bn, parity)] = rs
    return out


def _layout(plan):
    movw, live, S = plan
    slots = [(bn, j) for bn in range(N) for j in range(S[bn])]
    nslot = len(slots)
    movoff = np.concatenate([[0], np.cumsum(movw)]).astype(int)
    statw = nslot * TILE
    inw = statw + int(movoff[-1])
    nbins = tuple(w // W for w in movw)
    binoff = np.concatenate(
        [[0], np.cumsum([nbins[bn] for bn, _ in slots])]).astype(int)
    return slots, nslot, movoff, statw, inw, nbins, binoff


@lru_cache(maxsize=4)
def _build_program(plan):
    from concourse.bass import Bass
    from concourse.tile import TileContext
    import concourse.mybir as mybir

    f32 = mybir.dt.float32
    f16 = mybir.dt.float16

    movw, live, S = plan
    slots, nslot, movoff, statw, inw, nbins, binoff = _layout(plan)
    recipes = _recipes_of(movw)
    binw = int(binoff[-1])

    nc = Bass("TRN2", num_devices=N_CORES)

    inp_d = nc.dram_tensor("inp", [KROWS, inw], f16, kind="ExternalInput")
    bins_d = nc.dram_tensor("bins_out", [TILE, binw], f16,
                            kind="ExternalOutput")

    with TileContext(nc) as tc:
        with tc.tile_pool(name="const", bufs=1) as cpool, \
             tc.tile_pool(name="tree", bufs=3) as tpool, \
             tc.tile_pool(name="psum", bufs=2, space="PSUM") as ppool:
            inp_sb = cpool.tile([KROWS, inw], f16)
            # Warm up PE p-state and the Act engine off a tiny gpsimd memset.
            warm_in = cpool.tile([KROWS, TILE], f16)
            warm_sb = cpool.tile([TILE, 8], f16)
            nc.gpsimd.memset(warm_in[:, :], 0.0)
            wps = ppool.tile([TILE, GROUP], f32, tag="ps")
            nc.tensor.matmul(wps[:, 0:8], warm_in[:, 0:TILE],
                             warm_in[:, 0:8], start=True, stop=True)
            nc.scalar.activation(warm_sb, wps[:, 0:8],
                                 mybir.ActivationFunctionType.Copy)
            # Input DMA: each dma_start holds its engine's sequencer ~0.6us
            # and serializes on the global HWDGE, so keep only the critical
            # first-group pieces on the fast queues (Act casts must start
            # ASAP) and push all bulk input onto gpsimd's SWDGE path (the
            # Pool engine is idle and SWDGE doesn't contend with HWDGE).
            bsec = [statw + int(movoff[i]) for i in range(N + 1)]
            bf = slots[0][0] if nslot else 0
            s0 = min(2 * TILE, statw)
            h0 = min(bsec[bf] + GROUP, bsec[bf + 1])
            h1 = min(h0 + GROUP, bsec[bf + 1])
            nc.sync.dma_start(inp_sb[:, 0:s0], inp_d[:, 0:s0])
            nc.scalar.dma_start(inp_sb[:, bsec[bf]:h0], inp_d[:, bsec[bf]:h0])
            if h1 > h0:
                nc.sync.dma_start(inp_sb[:, h0:h1], inp_d[:, h0:h1])
            if statw > s0:
                nc.gpsimd.dma_start(inp_sb[:, s0:statw], inp_d[:, s0:statw])
            if bsec[bf + 1] > h1:
                nc.gpsimd.dma_start(inp_sb[:, h1:bsec[bf + 1]],
                                    inp_d[:, h1:bsec[bf + 1]])
            for i in range(N):
                if i == bf or bsec[i + 1] == bsec[i]:
                    continue
                mid = (bsec[i] + bsec[i + 1]) // 2
                nc.gpsimd.dma_start(inp_sb[:, bsec[i]:mid],
                                    inp_d[:, bsec[i]:mid])
                nc.gpsimd.dma_start(inp_sb[:, mid:bsec[i + 1]],
                                    inp_d[:, mid:bsec[i + 1]])
            stat_sb = inp_sb[:, 0:statw]

            bins_st = cpool.tile([TILE, binw], f16)
            drain_lo = [0]

            def drain(upto, last=False):
                if upto - drain_lo[0] >= 512 or (last and upto > drain_lo[0]):
                    nc.sync.dma_start(bins_d[:, drain_lo[0]:upto],
                                      bins_st[:, drain_lo[0]:upto])
                    drain_lo[0] = upto

            for s, (bn, j) in enumerate(slots):
                mov = inp_sb[:, bsec[bn]:bsec[bn + 1]]
                lhsT = stat_sb[:, s * TILE:(s + 1) * TILE]
                b0 = int(binoff[s])
                for gidx, (g0, gw, kind) in enumerate(recipes[(bn, j % 2)]):
                    bs = bins_st[:, b0 + g0 // W:b0 + (g0 + gw) // W]
                    ps = ppool.tile([TILE, GROUP], f32, tag="ps")
                    c0 = 0
                    while c0 < gw:
                        cw = min(CHUNK, gw - c0)
                        nc.tensor.matmul(
                            ps[:, c0:c0 + cw], lhsT,
                            mov[:, g0 + c0:g0 + c0 + cw],
                            start=True, stop=True)
                        c0 += cw
                    if kind == "r1":
                        nc.vector.reduce_max(
                            bs,
                            ps[:, 0:gw].rearrange("p (n w) -> p n w",
                                                  n=gw // W, w=W),
                            axis=mybir.AxisListType.X)
                        continue
                    h = gw // 2
                    cp = tpool.tile([TILE, GROUP], f16, tag="cp")
                    t1 = tpool.tile([TILE, GROUP // 2], f16, tag="t1")
                    if kind == "hc":
                        nc.scalar.activation(
                            cp[:, 0:h], ps[:, h:gw],
                            mybir.ActivationFunctionType.Copy)
                        nc.vector.tensor_max(t1[:, 0:h], ps[:, 0:h],
                                             cp[:, 0:h])
                    else:
                        nc.scalar.activation(
                            cp[:, 0:gw], ps[:, 0:gw],
                            mybir.ActivationFunctionType.Copy)
                        nc.vector.tensor_max(t1[:, 0:h], cp[:, 0:h],
                                             cp[:, h:gw])
                    t2 = tpool.tile([TILE, GROUP // 4], f16, tag="t2")
                    nc.vector.tensor_max(t2[:, 0:h // 2], t1[:, 0:h // 2],
                                         t1[:, h // 2:h])
                    t3 = tpool.tile([TILE, GROUP // 8], f16, tag="t3")
                    nc.vector.tensor_max(t3[:, 0:h // 4], t2[:, 0:h // 4],
                                         t2[:, h // 4:h // 2])
                    if gw >= 1024:
                        # finish with 2x tensor_max levels (cheaper than the
                        # 1x fp16 reduce); bins become residues mod nbins
                        t4 = tpool.tile([TILE, GROUP // 16], f16, tag="t4")
                        nc.vector.tensor_max(t4[:, 0:h // 8],
                                             t3[:, 0:h // 8],
                                             t3[:, h // 8:h // 4])
                        t5 = tpool.tile([TILE, GROUP // 32], f16, tag="t5")
                        nc.vector.tensor_max(t5[:, 0:h // 16],
                                             t4[:, 0:h // 16],
                                             t4[:, h // 16:h // 8])
                        nc.vector.tensor_max(bs, t5[:, 0:h // 32],
                                             t5[:, h // 32:h // 16])
                    else:
                        nc.vector.reduce_max(
                            bs,
                            t3[:, 0:h // 4].rearrange("p (n w) -> p n w",
                                                      n=gw // W, w=8),
                            axis=mybir.AxisListType.X)
                drain(int(binoff[s + 1]) - (int(binoff[s + 1]) % 512))
            drain(binw, last=True)

    # Walrus allows only ~1 sync wait per instruction; split extras onto
    # single-wait NoOps chained before it (same engine, program order).
    import concourse.mybir as mb
    fix = 0
    for fn in nc.m.functions:
        for blk in fn.blocks:
            insts = blk.instructions
            i = 0
            while i < len(insts):
                inst = insts[i]
                si = inst.sync_info
                if si is not None and len(si.on_wait) > 1:
                    head, last = si.on_wait[:-1], si.on_wait[-1:]
                    pre = []
                    for w in head:
                        fix += 1
                        nop = mb.InstNoOp(name=f"I-waitfix-{fix}", ins=[],
                                          outs=[])
                        nop.engine = inst.engine
                        nop.sync_info = mb.SyncInfo(on_wait=[w], on_update=[])
                        pre.append(nop)
                    si.on_wait = last
                    insts[i:i] = pre
                    i += len(pre)
                i += 1
    return nc


def _split16(x):
    h = x.astype(np.float16)
    l = (x - h.astype(np.float32)).astype(np.float16)
    return h, l


def _core_inputs(p1, p2, lengths2, core, lengths1=None):
    if lengths1 is None:
        lengths1 = np.full(N, P1, np.int32)
    plan = _plan_of(lengths1, lengths2)
    movw, live, S = plan
    slots, nslot, movoff, statw, inw, nbins, binoff = _layout(plan)

    inp = np.zeros((KROWS, inw), np.float16)
    stat = inp[:, 0:statw]
    for s, (bn, j) in enumerate(slots):
        g = j * N_CORES + core
        if g >= live[bn]:
            g = 0                              # dummy; host discards
        q0 = g * TILE
        p1n = p1[bn, q0:q0 + TILE]             # (128, 3)
        ah, al = _split16(p1n)
        sc = stat[:, s * TILE:(s + 1) * TILE]
        sc[0:3] = 2.0 * ah.T.astype(np.float32)
        sc[3:6] = 2.0 * ah.T.astype(np.float32)
        sc[6:9] = 2.0 * al.T.astype(np.float32)
        sc[9:15] = -1.0
        sc[15] = -1.0
    for bn in range(N):
        wb = movw[bn]
        L2 = int(lengths2[bn])
        mov = inp[:, statw + int(movoff[bn]):statw + int(movoff[bn + 1])]
        p2n = np.zeros((wb, D), np.float32)
        p2n[:L2] = p2[bn, :L2]
        bh, bl = _split16(p2n)
        ch, cl = _split16(p2n * p2n)
        mov[0:3] = bh.T                        # pairs with 2*ah
        mov[3:6] = bl.T                        # pairs with 2*ah
        mov[6:9] = bh.T                        # pairs with 2*al
        mov[9:12] = ch.T                       # pairs with -1
        mov[12:15] = cl.T                      # pairs with -1
        msk = np.zeros(wb, np.float16)
        msk[L2:] = BIGM
        mov[15] = msk                          # pairs with -1
    return {"inp": inp}


def _bin_cols_tables(movw):
    recipes = _recipes_of(movw)
    tables = {}
    for bn in range(N):
        for parity in (0, 1):
            rows = []
            for (g0, gw, kind) in recipes[(bn, parity)]:
                nb = gw // W
                if kind == "r1":
                    for b in range(nb):
                        rows.append(g0 + 64 * b
                                    + np.arange(64, dtype=np.int32))
                elif gw >= 1024:
                    # 6-level tensor_max tree: bin b = cols == b (mod nb)
                    for b in range(nb):
                        rows.append(g0 + b
                                    + nb * np.arange(64, dtype=np.int32))
                else:
                    step = gw // 8
                    offs = (np.arange(8, dtype=np.int32)[:, None]
                            + step * np.arange(8, dtype=np.int32)[None, :]
                            ).reshape(-1)
                    for b in range(nb):
                        rows.append(g0 + 8 * b + offs)
            tables[(bn, parity)] = np.stack(rows, axis=0)
    return tables


def kernel(p1, p2, lengths1, lengths2):
    from concourse.bass_utils import run_bass_kernel_spmd

    p1 = np.asarray(p1, np.float32)
    p2 = np.asarray(p2, np.float32)
    lengths1 = np.asarray(lengths1, np.int32)
    lengths2 = np.asarray(lengths2, np.int32)

    plan = _plan_of(lengths1, lengths2)
    movw, live, S = plan
    slots, nslot, movoff, statw, inw, nbins, binoff = _layout(plan)
    nc = _build_program(plan)
    in_maps = [_core_inputs(p1, p2, lengths2, c, lengths1)
               for c in range(N_CORES)]
    res = run_bass_kernel_spmd(nc, in_maps, core_ids=list(range(N_CORES)))

    tables = _bin_cols_tables(movw)

    dists = np.zeros((N, P1, K), np.float32)
    idx = np.zeros((N, P1, K), np.int64)

    # collect per-batch fp16 bin rows for all live tiles
    binvals = [np.zeros((live[bn] * TILE, nbins[bn]), np.float16)
               for bn in range(N)]
    for c in range(N_CORES):
        bv = res.results[c]["bins_out"]                  # (128, binw) fp16
        for s, (bn, j) in enumerate(slots):
            g = j * N_CORES + c
            if g >= live[bn]:
                continue
            q0 = g * TILE
            binvals[bn][q0:q0 + TILE] = bv[:, int(binoff[s]):int(binoff[s + 1])]

    RB = TILE * N_CORES        # one slot-row block = one recipe parity
    for bn in range(N):
        L1 = int(lengths1[bn])
        L2 = int(lengths2[bn])
        rows = min(live[bn] * TILE, P1)
        nb = nbins[bn]
        a = p1[bn]
        p2f = p2[bn]
        p1sq = (a[:, 0] * a[:, 0] + a[:, 1] * a[:, 1]) + a[:, 2] * a[:, 2]
        p2sq = (p2f[:, 0] * p2f[:, 0] + p2f[:, 1] * p2f[:, 1]) \
            + p2f[:, 2] * p2f[:, 2]
        bv = binvals[bn][:rows].astype(np.float32)       # (rows, nb)
        # select all bins >= 16th-largest bin value, capped at BIN_CAP
        order = np.argsort(-bv, axis=1, kind="stable")[:, :BIN_CAP]
        oval = np.take_along_axis(bv, order, axis=1)
        tau = oval[:, K - 1:K]                           # 16th largest value
        # bins beyond position 16 that tie tau stay selected (within cap);
        # mark unselected ones to point at bin 0 with +inf handled later
        selmask = oval >= tau                            # (rows, BIN_CAP)
        # rows where even position BIN_CAP-1 still ties tau may be truncated
        overflow = oval[:, BIN_CAP - 1] >= tau[:, 0]
        for r0 in range(0, rows, RB):
            r1_ = min(r0 + RB, rows)
            nr = r1_ - r0
            table = tables[(bn, (r0 // (TILE * N_CORES)) % 2)]
            cols = table[order[r0:r1_]].reshape(nr, BIN_CAP * W)
            colsc = np.minimum(cols, P2 - 1)
            cand = p2f[colsc]                            # (nr, C, 3)
            dot = np.einsum("rd,rcd->rc", a[r0:r1_], cand,
                            optimize=True).astype(np.float32)
            dcand = (p1sq[r0:r1_, None] + p2sq[colsc]
                     - 2.0 * dot).astype(np.float32)
            dcand[cols >= L2] = np.inf
            dcand[~np.repeat(selmask[r0:r1_], W, axis=1)] = np.inf
            part = np.argpartition(dcand, K + 8, axis=1)[:, :K + 8]
            dpart = np.take_along_axis(dcand, part, axis=1)
            cpart = np.take_along_axis(colsc, part, axis=1)
            ordv = np.lexsort((cpart, dpart), axis=1)[:, :K]
            idx[bn, r0:r1_] = np.take_along_axis(cpart, ordv, axis=1)
            dists[bn, r0:r1_] = np.take_along_axis(dpart, ordv, axis=1)
        # slow path: rows whose tie set exceeded the cap -> exact recompute
        for r in np.nonzero(overflow)[0]:
            d = p1sq[r] + p2sq - 2.0 * (p2f @ a[r])
            d = d.astype(np.float32)
            d[L2:] = np.inf
            o = np.lexsort((np.arange(P2), d))[:K]
            idx[bn, r] = o
            dists[bn, r] = d[o]
        dists[bn][~np.isfinite(dists[bn])] = 0.0
        dists[bn, L1:] = 0.0
        idx[bn, L1:] = 0
    return idx, dists


# revision 74
# speedup vs baseline: 1.0094x; 1.0094x over previous
"""KNN top-16 kernel for Trainium2 (8 NeuronCores, SPMD) — v10 (fp16 tree).

Problem (hardcoded): p1 (4,8192,3) f32, p2 (4,8192,3) f32, lengths1/2 (4,) i32.
Returns (idx int64 (4,8192,16), dists f32 (4,8192,16)) matching
jax.lax.top_k(-sq_dists, 16) semantics with PyTorch3D-style padding.

v10 pipeline per 2048-column PSUM group (per 128-query slot):
  PE   : fp16 hi/lo split matmul (16 contraction rows) -> fp32 PSUM,
          1 cycle/column (vs 4 for fp32), exact to ~1e-5 absolute.
  Act  : cast-copy PSUM fp32 -> SBUF fp16 (the Act engine is otherwise
          idle), feeding
  DVE  : a 3-level pairwise fp16 tensor_max tree (2x DVE mode) + one W=8
          fp16 tensor_reduce -> 64-column bin maxima, ~0.71 ns/elem instead
          of 1.04 for a direct fp32 reduce.
  Per-group recipes ('r1' = direct fp32 PSUM reduce with no Act stage,
  'hc' = Act casts only half and DVE's tree level 1 maxes the PSUM half
  against it, 'tree' = full cast) are assigned per (batch, slot parity) in
  _kind_of to balance Act vs DVE busy time globally AND locally; the
  measured split runs both engines at ~92% occupancy.
  The per-query top-16-bin selection runs on the HOST from the fp16 bins
  (monotone rounding keeps the coverage guarantee: a column among the true
  top-16 has at most 15 bins with a strictly larger bin max, so selecting
  all bins >= the 16th-largest bin value always covers it; rows whose
  tie-set exceeds the 32-bin cap fall back to an exact full-row recompute).
  The host then re-ranks the <=32x64 candidate columns exactly in fp32
  (reference formula + tie-break by lower index).
  Input DMA rides gpsimd's SWDGE queue except the first two groups (each
  dma_start holds its engine ~0.6us and serializes on the global HWDGE).

Sharding: live query tile g of batch n runs on core g%8, slot g//8.
"""

import numpy as np
from functools import lru_cache

N, P1, P2, D, K = 4, 8192, 8192, 3, 16
N_CORES = 8
TILE = 128             # query rows per tile
CHUNK = 512            # matmul free-dim chunk (one PSUM bank)
W = 32                 # columns per bin
GROUP = 2048           # psum group (4 banks)
KROWS = 16             # contraction rows (fp16 split encoding)
def _kind_of(bn, gi, parity):
    """'r1': direct fp32 PSUM reduce (no Act); 'hc': Act casts only the
    second half, DVE's tree level 1 maxes PSUM half vs cast half (same bin
    layout as 'tree'); 'tree': full Act cast + fp16 tree."""
    if bn == 0:
        return "r1" if gi == 1 else "tree"
    if bn == 1:
        return "r1" if gi == 2 else "tree"
    if bn == 2:
        if parity == 1:
            return "r1" if gi == 0 else "tree"
        return "hc" if gi == 0 else "tree"
    return "hc" if gi == 1 else "tree"
BIGM = np.float32(60000.0)   # mask magnitude (fits fp16)
BIN_CAP = 32           # host-side max selected bins per row before slow path


def _plan_of(lengths1, lengths2):
    movw = tuple(-(-int(l) // W) * W for l in lengths2)        # pad to bins
    live = tuple(min(P1 // TILE, -(-int(l) // TILE)) for l in lengths1)
    S = tuple(-(-lv // N_CORES) for lv in live)
    return (movw, live, S)


def _groups_of(wb):
    """[(g0, gw)] covering [0, wb) in GROUP-sized pieces."""
    gs = []
    g0 = 0
    while g0 < wb:
        gw = min(GROUP, wb - g0)
        gs.append((g0, gw))
        g0 += gw
    return gs


def _recipes_of(movw):
    """Per (batch, slot-parity): list of (g0, gw, kind)."""
    out = {}
    for bn in range(N):
        for parity in (0, 1):
            rs = []
            for gi, (g0, gw) in enumerate(_groups_of(movw[bn])):
                if gw == GROUP:
                    kind = _kind_of(bn, gi, parity)
                else:
                    kind = "r1" if gw <= 512 else "tree"
                rs.append((g0, gw, kind))
            out[(bn, parity)] = rs
    return out


def _layout(plan):
    movw, live, S = plan
    slots = [(bn, j) for bn in range(N) for j in range(S[bn])]
    nslot = len(slots)
    movoff = np.concatenate([[0], np.cumsum(movw)]).astype(int)
    statw = nslot * TILE
    inw = statw + int(movoff[-1])
    nbins = tuple(w // W for w in movw)
    binoff = np.concatenate(
        [[0], np.cumsum([nbins[bn] for bn, _ in slots])]).astype(int)
    return slots, nslot, movoff, statw, inw, nbins, binoff


@lru_cache(maxsize=4)
def _build_program(plan):
    from concourse.bass import Bass
    from concourse.tile import TileContext
    import concourse.mybir as mybir

    f32 = mybir.dt.float32
    f16 = mybir.dt.float16

    movw, live, S = plan
    slots, nslot, movoff, statw, inw, nbins, binoff = _layout(plan)
    recipes = _recipes_of(movw)
    binw = int(binoff[-1])

    nc = Bass("TRN2", num_devices=N_CORES)

    inp_d = nc.dram_tensor("inp", [KROWS, inw], f16, kind="ExternalInput")
    bins_d = nc.dram_tensor("bins_out", [TILE, binw], f16,
                            kind="ExternalOutput")

    with TileContext(nc) as tc:
        with tc.tile_pool(name="const", bufs=1) as cpool, \
             tc.tile_pool(name="tree", bufs=3) as tpool, \
             tc.tile_pool(name="psum", bufs=2, space="PSUM") as ppool:
            inp_sb = cpool.tile([KROWS, inw], f16)
            # Warm up PE p-state and the Act engine off a tiny gpsimd memset.
            warm_in = cpool.tile([KROWS, TILE], f16)
            warm_sb = cpool.tile([TILE, 8], f16)
            nc.gpsimd.memset(warm_in[:, :], 0.0)
            wps = ppool.tile([TILE, GROUP], f32, tag="ps")
            nc.tensor.matmul(wps[:, 0:8], warm_in[:, 0:TILE],
                             warm_in[:, 0:8], start=True, stop=True)
            nc.scalar.activation(warm_sb, wps[:, 0:8],
                                 mybir.ActivationFunctionType.Copy)
            # Input DMA: each dma_start holds its engine's sequencer ~0.6us
            # and serializes on the global HWDGE, so keep only the critical
            # first-group pieces on the fast queues (Act casts must start
            # ASAP) and push all bulk input onto gpsimd's SWDGE path (the
            # Pool engine is idle and SWDGE doesn't contend with HWDGE).
            bsec = [statw + int(movoff[i]) for i in range(N + 1)]
            bf = slots[0][0] if nslot else 0
            s0 = min(2 * TILE, statw)
            h0 = min(bsec[bf] + GROUP, bsec[bf + 1])
            h1 = min(h0 + GROUP, bsec[bf + 1])
            nc.sync.dma_start(inp_sb[:, 0:s0], inp_d[:, 0:s0])
            nc.scalar.dma_start(inp_sb[:, bsec[bf]:h0], inp_d[:, bsec[bf]:h0])
            if h1 > h0:
                nc.sync.dma_start(inp_sb[:, h0:h1], inp_d[:, h0:h1])
            if statw > s0:
                nc.gpsimd.dma_start(inp_sb[:, s0:statw], inp_d[:, s0:statw])
            if bsec[bf + 1] > h1:
                nc.gpsimd.dma_start(inp_sb[:, h1:bsec[bf + 1]],
                                    inp_d[:, h1:bsec[bf + 1]])
            for i in range(N):
                if i == bf or bsec[i + 1] == bsec[i]:
                    continue
                mid = (bsec[i] + bsec[i + 1]) // 2
                nc.gpsimd.dma_start(inp_sb[:, bsec[i]:mid],
                                    inp_d[:, bsec[i]:mid])
                nc.gpsimd.dma_start(inp_sb[:, mid:bsec[i + 1]],
                                    inp_d[:, mid:bsec[i + 1]])
            stat_sb = inp_sb[:, 0:statw]

            bins_st = cpool.tile([TILE, binw], f16)
            drain_lo = [0]

            def drain(upto, last=False):
                if upto - drain_lo[0] >= 512 or (last and upto > drain_lo[0]):
                    nc.sync.dma_start(bins_d[:, drain_lo[0]:upto],
                                      bins_st[:, drain_lo[0]:upto])
                    drain_lo[0] = upto

            for s, (bn, j) in enumerate(slots):
                mov = inp_sb[:, bsec[bn]:bsec[bn + 1]]
                lhsT = stat_sb[:, s * TILE:(s + 1) * TILE]
                b0 = int(binoff[s])
                for gidx, (g0, gw, kind) in enumerate(recipes[(bn, j % 2)]):
                    bs = bins_st[:, b0 + g0 // W:b0 + (g0 + gw) // W]
                    ps = ppool.tile([TILE, GROUP], f32, tag="ps")
                    c0 = 0
                    while c0 < gw:
                        cw = min(CHUNK, gw - c0)
                        nc.tensor.matmul(
                            ps[:, c0:c0 + cw], lhsT,
                            mov[:, g0 + c0:g0 + c0 + cw],
                            start=True, stop=True)
                        c0 += cw
                    if kind == "r1":
                        nc.vector.reduce_max(
                            bs,
                            ps[:, 0:gw].rearrange("p (n w) -> p n w",
                                                  n=gw // W, w=W),
                            axis=mybir.AxisListType.X)
                        continue
                    h = gw // 2
                    cp = tpool.tile([TILE, GROUP], f16, tag="cp")
                    t1 = tpool.tile([TILE, GROUP // 2], f16, tag="t1")
                    if kind == "hc":
                        nc.scalar.activation(
                            cp[:, 0:h], ps[:, h:gw],
                            mybir.ActivationFunctionType.Copy)
                        nc.vector.tensor_max(t1[:, 0:h], ps[:, 0:h],
                                             cp[:, 0:h])
                    else:
                        nc.scalar.activation(
                            cp[:, 0:gw], ps[:, 0:gw],
                            mybir.ActivationFunctionType.Copy)
                        nc.vector.tensor_max(t1[:, 0:h], cp[:, 0:h],
                                             cp[:, h:gw])
                    t2 = tpool.tile([TILE, GROUP // 4], f16, tag="t2")
                    nc.vector.tensor_max(t2[:, 0:h // 2], t1[:, 0:h // 2],
                                         t1[:, h // 2:h])
                    t3 = tpool.tile([TILE, GROUP // 8], f16, tag="t3")
                    nc.vector.tensor_max(t3[:, 0:h // 4], t2[:, 0:h // 4],
                                         t2[:, h // 4:h // 2])
                    if gw >= 1024:
                        # finish with 2x tensor_max levels (cheaper than the
                        # 1x fp16 reduce); bins become residues mod nbins
                        t4 = tpool.tile([TILE, GROUP // 16], f16, tag="t4")
                        nc.vector.tensor_max(t4[:, 0:h // 8],
                                             t3[:, 0:h // 8],
                                             t3[:, h // 8:h // 4])
                        nc.vector.tensor_max(bs, t4[:, 0:h // 16],
                                             t4[:, h // 16:h // 8])
                    else:
                        nc.vector.reduce_max(
                            bs,
                            t3[:, 0:h // 4].rearrange("p (n w) -> p n w",
                                                      n=gw // W, w=W // 8),
                            axis=mybir.AxisListType.X)
                drain(int(binoff[s + 1]) - (int(binoff[s + 1]) % 512))
            drain(binw, last=True)

    # Walrus allows only ~1 sync wait per instruction; split extras onto
    # single-wait NoOps chained before it (same engine, program order).
    import concourse.mybir as mb
    fix = 0
    for fn in nc.m.functions:
        for blk in fn.blocks:
            insts = blk.instructions
            i = 0
            while i < len(insts):
                inst = insts[i]
                si = inst.sync_info
                if si is not None and len(si.on_wait) > 1:
                    head, last = si.on_wait[:-1], si.on_wait[-1:]
                    pre = []
                    for w in head:
                        fix += 1
                        nop = mb.InstNoOp(name=f"I-waitfix-{fix}", ins=[],
                                          outs=[])
                        nop.engine = inst.engine
                        nop.sync_info = mb.SyncInfo(on_wait=[w], on_update=[])
                        pre.append(nop)
                    si.on_wait = last
                    insts[i:i] = pre
                    i += len(pre)
                i += 1
    return nc


def _split16(x):
    h = x.astype(np.float16)
    l = (x - h.astype(np.float32)).astype(np.float16)
    return h, l


def _core_inputs(p1, p2, lengths2, core, lengths1=None):
    if lengths1 is None:
        lengths1 = np.full(N, P1, np.int32)
    plan = _plan_of(lengths1, lengths2)
    movw, live, S = plan
    slots, nslot, movoff, statw, inw, nbins, binoff = _layout(plan)

    inp = np.zeros((KROWS, inw), np.float16)
    stat = inp[:, 0:statw]
    for s, (bn, j) in enumerate(slots):
        g = j * N_CORES + core
        if g >= live[bn]:
            g = 0                              # dummy; host discards
        q0 = g * TILE
        p1n = p1[bn, q0:q0 + TILE]             # (128, 3)
        ah, al = _split16(p1n)
        sc = stat[:, s * TILE:(s + 1) * TILE]
        sc[0:3] = 2.0 * ah.T.astype(np.float32)
        sc[3:6] = 2.0 * ah.T.astype(np.float32)
        sc[6:9] = 2.0 * al.T.astype(np.float32)
        sc[9:15] = -1.0
        sc[15] = -1.0
    for bn in range(N):
        wb = movw[bn]
        L2 = int(lengths2[bn])
        mov = inp[:, statw + int(movoff[bn]):statw + int(movoff[bn + 1])]
        p2n = np.zeros((wb, D), np.float32)
        p2n[:L2] = p2[bn, :L2]
        bh, bl = _split16(p2n)
        ch, cl = _split16(p2n * p2n)
        mov[0:3] = bh.T                        # pairs with 2*ah
        mov[3:6] = bl.T                        # pairs with 2*ah
        mov[6:9] = bh.T                        # pairs with 2*al
        mov[9:12] = ch.T                       # pairs with -1
        mov[12:15] = cl.T                      # pairs with -1
        msk = np.zeros(wb, np.float16)
        msk[L2:] = BIGM
        mov[15] = msk                          # pairs with -1
    return {"inp": inp}


def _bin_cols_tables(movw):
    recipes = _recipes_of(movw)
    tables = {}
    for bn in range(N):
        for parity in (0, 1):
            rows = []
            for (g0, gw, kind) in recipes[(bn, parity)]:
                nb = gw // W
                if kind == "r1":
                    for b in range(nb):
                        rows.append(g0 + W * b
                                    + np.arange(W, dtype=np.int32))
                elif gw >= 1024:
                    # tensor_max tree: bin b = cols == b (mod nb)
                    for b in range(nb):
                        rows.append(g0 + b
                                    + nb * np.arange(W, dtype=np.int32))
                else:
                    step = gw // 8
                    offs = (np.arange(8, dtype=np.int32)[:, None]
                            + step * np.arange(8, dtype=np.int32)[None, :]
                            ).reshape(-1)
                    for b in range(nb):
                        rows.append(g0 + 8 * b + offs)
            tables[(bn, parity)] = np.stack(rows, axis=0)
    return tables


def kernel(p1, p2, lengths1, lengths2):
    from concourse.bass_utils import run_bass_kernel_spmd

    p1 = np.asarray(p1, np.float32)
    p2 = np.asarray(p2, np.float32)
    lengths1 = np.asarray(lengths1, np.int32)
    lengths2 = np.asarray(lengths2, np.int32)

    plan = _plan_of(lengths1, lengths2)
    movw, live, S = plan
    slots, nslot, movoff, statw, inw, nbins, binoff = _layout(plan)
    nc = _build_program(plan)
    in_maps = [_core_inputs(p1, p2, lengths2, c, lengths1)
               for c in range(N_CORES)]
    res = run_bass_kernel_spmd(nc, in_maps, core_ids=list(range(N_CORES)))

    tables = _bin_cols_tables(movw)

    dists = np.zeros((N, P1, K), np.float32)
    idx = np.zeros((N, P1, K), np.int64)

    # collect per-batch fp16 bin rows for all live tiles
    binvals = [np.zeros((live[bn] * TILE, nbins[bn]), np.float16)
               for bn in range(N)]
    for c in range(N_CORES):
        bv = res.results[c]["bins_out"]                  # (128, binw) fp16
        for s, (bn, j) in enumerate(slots):
            g = j * N_CORES + c
            if g >= live[bn]:
                continue
            q0 = g * TILE
            binvals[bn][q0:q0 + TILE] = bv[:, int(binoff[s]):int(binoff[s + 1])]

    RB = TILE * N_CORES        # one slot-row block = one recipe parity
    for bn in range(N):
        L1 = int(lengths1[bn])
        L2 = int(lengths2[bn])
        rows = min(live[bn] * TILE, P1)
        nb = nbins[bn]
        a = p1[bn]
        p2f = p2[bn]
        p1sq = (a[:, 0] * a[:, 0] + a[:, 1] * a[:, 1]) + a[:, 2] * a[:, 2]
        p2sq = (p2f[:, 0] * p2f[:, 0] + p2f[:, 1] * p2f[:, 1]) \
            + p2f[:, 2] * p2f[:, 2]
        bv = binvals[bn][:rows].astype(np.float32)       # (rows, nb)
        # select all bins >= 16th-largest bin value, capped at BIN_CAP
        order = np.argsort(-bv, axis=1, kind="stable")[:, :BIN_CAP]
        oval = np.take_along_axis(bv, order, axis=1)
        tau = oval[:, K - 1:K]                           # 16th largest value
        # bins beyond position 16 that tie tau stay selected (within cap);
        # mark unselected ones to point at bin 0 with +inf handled later
        selmask = oval >= tau                            # (rows, BIN_CAP)
        # rows where even position BIN_CAP-1 still ties tau may be truncated
        overflow = oval[:, BIN_CAP - 1] >= tau[:, 0]
        for r0 in range(0, rows, RB):
            r1_ = min(r0 + RB, rows)
            nr = r1_ - r0
            table = tables[(bn, (r0 // (TILE * N_CORES)) % 2)]
            cols = table[order[r0:r1_]].reshape(nr, BIN_CAP * W)
            colsc = np.minimum(cols, P2 - 1)
            cand = p2f[colsc]                            # (nr, C, 3)
            dot = np.einsum("rd,rcd->rc", a[r0:r1_], cand,
                            optimize=True).astype(np.float32)
            dcand = (p1sq[r0:r1_, None] + p2sq[colsc]
                     - 2.0 * dot).astype(np.float32)
            dcand[cols >= L2] = np.inf
            dcand[~np.repeat(selmask[r0:r1_], W, axis=1)] = np.inf
            part = np.argpartition(dcand, K + 8, axis=1)[:, :K + 8]
            dpart = np.take_along_axis(dcand, part, axis=1)
            cpart = np.take_along_axis(colsc, part, axis=1)
            ordv = np.lexsort((cpart, dpart), axis=1)[:, :K]
            idx[bn, r0:r1_] = np.take_along_axis(cpart, ordv, axis=1)
            dists[bn, r0:r1_] = np.take_along_axis(dpart, ordv, axis=1)
        # slow path: rows whose tie set exceeded the cap -> exact recompute
        for r in np.nonzero(overflow)[0]:
            d = p1sq[r] + p2sq - 2.0 * (p2f @ a[r])
            d = d.astype(np.float32)
            d[L2:] = np.inf
            o = np.lexsort((np.arange(P2), d))[:K]
            idx[bn, r] = o
            dists[bn, r] = d[o]
        dists[bn][~np.isfinite(dists[bn])] = 0.0
        dists[bn, L1:] = 0.0
        idx[bn, L1:] = 0
    return idx, dists


# revision 75
# speedup vs baseline: 1.0130x; 1.0036x over previous
"""KNN top-16 kernel for Trainium2 (8 NeuronCores, SPMD) — v10 (fp16 tree).

Problem (hardcoded): p1 (4,8192,3) f32, p2 (4,8192,3) f32, lengths1/2 (4,) i32.
Returns (idx int64 (4,8192,16), dists f32 (4,8192,16)) matching
jax.lax.top_k(-sq_dists, 16) semantics with PyTorch3D-style padding.

v10 pipeline per 2048-column PSUM group (per 128-query slot):
  PE   : fp16 hi/lo split matmul (16 contraction rows) -> fp32 PSUM,
          1 cycle/column (vs 4 for fp32), exact to ~1e-5 absolute.
  Act  : cast-copy PSUM fp32 -> SBUF fp16 (the Act engine is otherwise
          idle), feeding
  DVE  : a 3-level pairwise fp16 tensor_max tree (2x DVE mode) + one W=8
          fp16 tensor_reduce -> 64-column bin maxima, ~0.71 ns/elem instead
          of 1.04 for a direct fp32 reduce.
  Per-group recipes ('r1' = direct fp32 PSUM reduce with no Act stage,
  'hc' = Act casts only half and DVE's tree level 1 maxes the PSUM half
  against it, 'tree' = full cast) are assigned per (batch, slot parity) in
  _kind_of to balance Act vs DVE busy time globally AND locally; the
  measured split runs both engines at ~92% occupancy.
  The per-query top-16-bin selection runs on the HOST from the fp16 bins
  (monotone rounding keeps the coverage guarantee: a column among the true
  top-16 has at most 15 bins with a strictly larger bin max, so selecting
  all bins >= the 16th-largest bin value always covers it; rows whose
  tie-set exceeds the 32-bin cap fall back to an exact full-row recompute).
  The host then re-ranks the <=32x64 candidate columns exactly in fp32
  (reference formula + tie-break by lower index).
  Input DMA rides gpsimd's SWDGE queue except the first two groups (each
  dma_start holds its engine ~0.6us and serializes on the global HWDGE).

Sharding: live query tile g of batch n runs on core g%8, slot g//8.
"""

import numpy as np
from functools import lru_cache

N, P1, P2, D, K = 4, 8192, 8192, 3, 16
N_CORES = 8
TILE = 128             # query rows per tile
CHUNK = 512            # matmul free-dim chunk (one PSUM bank)
W = 16                 # columns per bin
GROUP = 2048           # psum group (4 banks)
KROWS = 16             # contraction rows (fp16 split encoding)
def _kind_of(bn, gi, parity):
    """'r1': direct fp32 PSUM reduce (no Act); 'hc': Act casts only the
    second half, DVE's tree level 1 maxes PSUM half vs cast half (same bin
    layout as 'tree'); 'tree': full Act cast + fp16 tree."""
    if bn == 0:
        return "r1" if gi == 1 else "tree"
    if bn == 1:
        return "r1" if gi == 2 else "tree"
    if bn == 2:
        if parity == 1:
            return "r1" if gi == 0 else "tree"
        return "hc" if gi == 0 else "tree"
    return "hc" if gi == 1 else "tree"
BIGM = np.float32(60000.0)   # mask magnitude (fits fp16)
BIN_CAP = 32           # host-side max selected bins per row before slow path


def _plan_of(lengths1, lengths2):
    movw = tuple(-(-int(l) // W) * W for l in lengths2)        # pad to bins
    live = tuple(min(P1 // TILE, -(-int(l) // TILE)) for l in lengths1)
    S = tuple(-(-lv // N_CORES) for lv in live)
    return (movw, live, S)


def _groups_of(wb):
    """[(g0, gw)] covering [0, wb) in GROUP-sized pieces."""
    gs = []
    g0 = 0
    while g0 < wb:
        gw = min(GROUP, wb - g0)
        gs.append((g0, gw))
        g0 += gw
    return gs


def _recipes_of(movw):
    """Per (batch, slot-parity): list of (g0, gw, kind)."""
    out = {}
    for bn in range(N):
        for parity in (0, 1):
            rs = []
            for gi, (g0, gw) in enumerate(_groups_of(movw[bn])):
                if gw == GROUP:
                    kind = _kind_of(bn, gi, parity)
                else:
                    kind = "r1" if gw <= 512 else "tree"
                rs.append((g0, gw, kind))
            out[(bn, parity)] = rs
    return out


def _layout(plan):
    movw, live, S = plan
    slots = [(bn, j) for bn in range(N) for j in range(S[bn])]
    nslot = len(slots)
    movoff = np.concatenate([[0], np.cumsum(movw)]).astype(int)
    statw = nslot * TILE
    inw = statw + int(movoff[-1])
    nbins = tuple(w // W for w in movw)
    binoff = np.concatenate(
        [[0], np.cumsum([nbins[bn] for bn, _ in slots])]).astype(int)
    return slots, nslot, movoff, statw, inw, nbins, binoff


@lru_cache(maxsize=4)
def _build_program(plan):
    from concourse.bass import Bass
    from concourse.tile import TileContext
    import concourse.mybir as mybir

    f32 = mybir.dt.float32
    f16 = mybir.dt.float16

    movw, live, S = plan
    slots, nslot, movoff, statw, inw, nbins, binoff = _layout(plan)
    recipes = _recipes_of(movw)
    binw = int(binoff[-1])

    nc = Bass("TRN2", num_devices=N_CORES)

    inp_d = nc.dram_tensor("inp", [KROWS, inw], f16, kind="ExternalInput")
    bins_d = nc.dram_tensor("bins_out", [TILE, binw], f16,
                            kind="ExternalOutput")

    with TileContext(nc) as tc:
        with tc.tile_pool(name="const", bufs=1) as cpool, \
             tc.tile_pool(name="tree", bufs=3) as tpool, \
             tc.tile_pool(name="psum", bufs=2, space="PSUM") as ppool:
            inp_sb = cpool.tile([KROWS, inw], f16)
            # Warm up PE p-state and the Act engine off a tiny gpsimd memset.
            warm_in = cpool.tile([KROWS, TILE], f16)
            warm_sb = cpool.tile([TILE, 8], f16)
            nc.gpsimd.memset(warm_in[:, :], 0.0)
            wps = ppool.tile([TILE, GROUP], f32, tag="ps")
            nc.tensor.matmul(wps[:, 0:8], warm_in[:, 0:TILE],
                             warm_in[:, 0:8], start=True, stop=True)
            nc.scalar.activation(warm_sb, wps[:, 0:8],
                                 mybir.ActivationFunctionType.Copy)
            # Input DMA: each dma_start holds its engine's sequencer ~0.6us
            # and serializes on the global HWDGE, so keep only the critical
            # first-group pieces on the fast queues (Act casts must start
            # ASAP) and push all bulk input onto gpsimd's SWDGE path (the
            # Pool engine is idle and SWDGE doesn't contend with HWDGE).
            bsec = [statw + int(movoff[i]) for i in range(N + 1)]
            bf = slots[0][0] if nslot else 0
            s0 = min(2 * TILE, statw)
            h0 = min(bsec[bf] + GROUP, bsec[bf + 1])
            h1 = min(h0 + GROUP, bsec[bf + 1])
            nc.sync.dma_start(inp_sb[:, 0:s0], inp_d[:, 0:s0])
            nc.scalar.dma_start(inp_sb[:, bsec[bf]:h0], inp_d[:, bsec[bf]:h0])
            if h1 > h0:
                nc.sync.dma_start(inp_sb[:, h0:h1], inp_d[:, h0:h1])
            if statw > s0:
                nc.gpsimd.dma_start(inp_sb[:, s0:statw], inp_d[:, s0:statw])
            if bsec[bf + 1] > h1:
                nc.gpsimd.dma_start(inp_sb[:, h1:bsec[bf + 1]],
                                    inp_d[:, h1:bsec[bf + 1]])
            for i in range(N):
                if i == bf or bsec[i + 1] == bsec[i]:
                    continue
                mid = (bsec[i] + bsec[i + 1]) // 2
                nc.gpsimd.dma_start(inp_sb[:, bsec[i]:mid],
                                    inp_d[:, bsec[i]:mid])
                nc.gpsimd.dma_start(inp_sb[:, mid:bsec[i + 1]],
                                    inp_d[:, mid:bsec[i + 1]])
            stat_sb = inp_sb[:, 0:statw]

            bins_st = cpool.tile([TILE, binw], f16)
            drain_lo = [0]

            def drain(upto, last=False):
                if upto - drain_lo[0] >= 512 or (last and upto > drain_lo[0]):
                    nc.sync.dma_start(bins_d[:, drain_lo[0]:upto],
                                      bins_st[:, drain_lo[0]:upto])
                    drain_lo[0] = upto

            for s, (bn, j) in enumerate(slots):
                mov = inp_sb[:, bsec[bn]:bsec[bn + 1]]
                lhsT = stat_sb[:, s * TILE:(s + 1) * TILE]
                b0 = int(binoff[s])
                for gidx, (g0, gw, kind) in enumerate(recipes[(bn, j % 2)]):
                    bs = bins_st[:, b0 + g0 // W:b0 + (g0 + gw) // W]
                    ps = ppool.tile([TILE, GROUP], f32, tag="ps")
                    c0 = 0
                    while c0 < gw:
                        cw = min(CHUNK, gw - c0)
                        nc.tensor.matmul(
                            ps[:, c0:c0 + cw], lhsT,
                            mov[:, g0 + c0:g0 + c0 + cw],
                            start=True, stop=True)
                        c0 += cw
                    if kind == "r1":
                        nc.vector.reduce_max(
                            bs,
                            ps[:, 0:gw].rearrange("p (n w) -> p n w",
                                                  n=gw // W, w=W),
                            axis=mybir.AxisListType.X)
                        continue
                    h = gw // 2
                    cp = tpool.tile([TILE, GROUP], f16, tag="cp")
                    t1 = tpool.tile([TILE, GROUP // 2], f16, tag="t1")
                    if kind == "hc":
                        nc.scalar.activation(
                            cp[:, 0:h], ps[:, h:gw],
                            mybir.ActivationFunctionType.Copy)
                        nc.vector.tensor_max(t1[:, 0:h], ps[:, 0:h],
                                             cp[:, 0:h])
                    else:
                        nc.scalar.activation(
                            cp[:, 0:gw], ps[:, 0:gw],
                            mybir.ActivationFunctionType.Copy)
                        nc.vector.tensor_max(t1[:, 0:h], cp[:, 0:h],
                                             cp[:, h:gw])
                    t2 = tpool.tile([TILE, GROUP // 4], f16, tag="t2")
                    nc.vector.tensor_max(t2[:, 0:h // 2], t1[:, 0:h // 2],
                                         t1[:, h // 2:h])
                    t3 = tpool.tile([TILE, GROUP // 8], f16, tag="t3")
                    nc.vector.tensor_max(t3[:, 0:h // 4], t2[:, 0:h // 4],
                                         t2[:, h // 4:h // 2])
                    if gw >= 1024:
                        # finish with a 2x tensor_max level (cheaper than the
                        # 1x fp16 reduce); bins become residues mod nbins
                        nc.vector.tensor_max(bs, t3[:, 0:h // 8],
                                             t3[:, h // 8:h // 4])
                    else:
                        nc.vector.reduce_max(
                            bs,
                            t3[:, 0:h // 4].rearrange("p (n w) -> p n w",
                                                      n=gw // W, w=W // 8),
                            axis=mybir.AxisListType.X)
                drain(int(binoff[s + 1]) - (int(binoff[s + 1]) % 512))
            drain(binw, last=True)

    # Walrus allows only ~1 sync wait per instruction; split extras onto
    # single-wait NoOps chained before it (same engine, program order).
    import concourse.mybir as mb
    fix = 0
    for fn in nc.m.functions:
        for blk in fn.blocks:
            insts = blk.instructions
            i = 0
            while i < len(insts):
                inst = insts[i]
                si = inst.sync_info
                if si is not None and len(si.on_wait) > 1:
                    head, last = si.on_wait[:-1], si.on_wait[-1:]
                    pre = []
                    for w in head:
                        fix += 1
                        nop = mb.InstNoOp(name=f"I-waitfix-{fix}", ins=[],
                                          outs=[])
                        nop.engine = inst.engine
                        nop.sync_info = mb.SyncInfo(on_wait=[w], on_update=[])
                        pre.append(nop)
                    si.on_wait = last
                    insts[i:i] = pre
                    i += len(pre)
                i += 1
    return nc


def _split16(x):
    h = x.astype(np.float16)
    l = (x - h.astype(np.float32)).astype(np.float16)
    return h, l


def _core_inputs(p1, p2, lengths2, core, lengths1=None):
    if lengths1 is None:
        lengths1 = np.full(N, P1, np.int32)
    plan = _plan_of(lengths1, lengths2)
    movw, live, S = plan
    slots, nslot, movoff, statw, inw, nbins, binoff = _layout(plan)

    inp = np.zeros((KROWS, inw), np.float16)
    stat = inp[:, 0:statw]
    for s, (bn, j) in enumerate(slots):
        g = j * N_CORES + core
        if g >= live[bn]:
            g = 0                              # dummy; host discards
        q0 = g * TILE
        p1n = p1[bn, q0:q0 + TILE]             # (128, 3)
        ah, al = _split16(p1n)
        sc = stat[:, s * TILE:(s + 1) * TILE]
        sc[0:3] = 2.0 * ah.T.astype(np.float32)
        sc[3:6] = 2.0 * ah.T.astype(np.float32)
        sc[6:9] = 2.0 * al.T.astype(np.float32)
        sc[9:15] = -1.0
        sc[15] = -1.0
    for bn in range(N):
        wb = movw[bn]
        L2 = int(lengths2[bn])
        mov = inp[:, statw + int(movoff[bn]):statw + int(movoff[bn + 1])]
        p2n = np.zeros((wb, D), np.float32)
        p2n[:L2] = p2[bn, :L2]
        bh, bl = _split16(p2n)
        ch, cl = _split16(p2n * p2n)
        mov[0:3] = bh.T                        # pairs with 2*ah
        mov[3:6] = bl.T                        # pairs with 2*ah
        mov[6:9] = bh.T                        # pairs with 2*al
        mov[9:12] = ch.T                       # pairs with -1
        mov[12:15] = cl.T                      # pairs with -1
        msk = np.zeros(wb, np.float16)
        msk[L2:] = BIGM
        mov[15] = msk                          # pairs with -1
    return {"inp": inp}


def _bin_cols_tables(movw):
    recipes = _recipes_of(movw)
    tables = {}
    for bn in range(N):
        for parity in (0, 1):
            rows = []
            for (g0, gw, kind) in recipes[(bn, parity)]:
                nb = gw // W
                if kind == "r1":
                    for b in range(nb):
                        rows.append(g0 + W * b
                                    + np.arange(W, dtype=np.int32))
                elif gw >= 1024:
                    # tensor_max tree: bin b = cols == b (mod nb)
                    for b in range(nb):
                        rows.append(g0 + b
                                    + nb * np.arange(W, dtype=np.int32))
                else:
                    step = gw // 8
                    offs = (np.arange(8, dtype=np.int32)[:, None]
                            + step * np.arange(8, dtype=np.int32)[None, :]
                            ).reshape(-1)
                    for b in range(nb):
                        rows.append(g0 + 8 * b + offs)
            tables[(bn, parity)] = np.stack(rows, axis=0)
    return tables


def kernel(p1, p2, lengths1, lengths2):
    from concourse.bass_utils import run_bass_kernel_spmd

    p1 = np.asarray(p1, np.float32)
    p2 = np.asarray(p2, np.float32)
    lengths1 = np.asarray(lengths1, np.int32)
    lengths2 = np.asarray(lengths2, np.int32)

    plan = _plan_of(lengths1, lengths2)
    movw, live, S = plan
    slots, nslot, movoff, statw, inw, nbins, binoff = _layout(plan)
    nc = _build_program(plan)
    in_maps = [_core_inputs(p1, p2, lengths2, c, lengths1)
               for c in range(N_CORES)]
    res = run_bass_kernel_spmd(nc, in_maps, core_ids=list(range(N_CORES)))

    tables = _bin_cols_tables(movw)

    dists = np.zeros((N, P1, K), np.float32)
    idx = np.zeros((N, P1, K), np.int64)

    # collect per-batch fp16 bin rows for all live tiles
    binvals = [np.zeros((live[bn] * TILE, nbins[bn]), np.float16)
               for bn in range(N)]
    for c in range(N_CORES):
        bv = res.results[c]["bins_out"]                  # (128, binw) fp16
        for s, (bn, j) in enumerate(slots):
            g = j * N_CORES + c
            if g >= live[bn]:
                continue
            q0 = g * TILE
            binvals[bn][q0:q0 + TILE] = bv[:, int(binoff[s]):int(binoff[s + 1])]

    RB = TILE * N_CORES        # one slot-row block = one recipe parity
    for bn in range(N):
        L1 = int(lengths1[bn])
        L2 = int(lengths2[bn])
        rows = min(live[bn] * TILE, P1)
        nb = nbins[bn]
        a = p1[bn]
        p2f = p2[bn]
        p1sq = (a[:, 0] * a[:, 0] + a[:, 1] * a[:, 1]) + a[:, 2] * a[:, 2]
        p2sq = (p2f[:, 0] * p2f[:, 0] + p2f[:, 1] * p2f[:, 1]) \
            + p2f[:, 2] * p2f[:, 2]
        bv = binvals[bn][:rows].astype(np.float32)       # (rows, nb)
        # select all bins >= 16th-largest bin value, capped at BIN_CAP
        order = np.argsort(-bv, axis=1, kind="stable")[:, :BIN_CAP]
        oval = np.take_along_axis(bv, order, axis=1)
        tau = oval[:, K - 1:K]                           # 16th largest value
        # bins beyond position 16 that tie tau stay selected (within cap);
        # mark unselected ones to point at bin 0 with +inf handled later
        selmask = oval >= tau                            # (rows, BIN_CAP)
        # rows where even position BIN_CAP-1 still ties tau may be truncated
        overflow = oval[:, BIN_CAP - 1] >= tau[:, 0]
        for r0 in range(0, rows, RB):
            r1_ = min(r0 + RB, rows)
            nr = r1_ - r0
            table = tables[(bn, (r0 // (TILE * N_CORES)) % 2)]
            cols = table[order[r0:r1_]].reshape(nr, BIN_CAP * W)
            colsc = np.minimum(cols, P2 - 1)
            cand = p2f[colsc]                            # (nr, C, 3)
            dot = np.einsum("rd,rcd->rc", a[r0:r1_], cand,
                            optimize=True).astype(np.float32)
            dcand = (p1sq[r0:r1_, None] + p2sq[colsc]
                     - 2.0 * dot).astype(np.float32)
            dcand[cols >= L2] = np.inf
            dcand[~np.repeat(selmask[r0:r1_], W, axis=1)] = np.inf
            part = np.argpartition(dcand, K + 8, axis=1)[:, :K + 8]
            dpart = np.take_along_axis(dcand, part, axis=1)
            cpart = np.take_along_axis(colsc, part, axis=1)
            ordv = np.lexsort((cpart, dpart), axis=1)[:, :K]
            idx[bn, r0:r1_] = np.take_along_axis(cpart, ordv, axis=1)
            dists[bn, r0:r1_] = np.take_along_axis(dpart, ordv, axis=1)
        # slow path: rows whose tie set exceeded the cap -> exact recompute
        for r in np.nonzero(overflow)[0]:
            d = p1sq[r] + p2sq - 2.0 * (p2f @ a[r])
            d = d.astype(np.float32)
            d[L2:] = np.inf
            o = np.lexsort((np.arange(P2), d))[:K]
            idx[bn, r] = o
            dists[bn, r] = d[o]
        dists[bn][~np.isfinite(dists[bn])] = 0.0
        dists[bn, L1:] = 0.0
        idx[bn, L1:] = 0
    return idx, dists
